# revision 38
# baseline (speedup 1.0000x reference)
"""Trainium2 Bass kernel for AdaptiveDiffusionBlock (8 NeuronCores, SPMD).

Row-shards N_P=2048 over 8 cores (256 rows each). v2 restructure:

    residual = Xf1@Wp0.T + Xf2@Wp1.T + Xa1@Wa0.T + Xa2@Wa1.T
    Xf1 = Rf@x, Xf2 = (Rf@Rf)@x = M3@x, Xa1 = attn0@x, Xa2 = (attn1@attn0)@x = M4@x

Two stationary-x sweeps: pass1 streams [rfT | M3T] (no attention dep),
pass2 streams [attn0T | M4T] with LayerNorm fused per column chunk.
M3T computed on-device from host-supplied row-major Rf (rf_sw); M4T from
an attn0 AllGather (4 MiB fp8 -- the only large collective; the v1 P/Q
allgathers, projections and stage-E are gone). Top-p thresholds via
4-iteration binary search on t in (0,1] (u = exp(l - rowmax)).

kernel(**inputs) takes full numpy inputs, returns the full output.
"""

import sys

for _p in ("/opt/trn_rl_repo", "/root/.axon_site", "/root/.axon_site/_ro/trn_rl_repo"):
    if _p not in sys.path:
        sys.path.append(_p)

import numpy as np
import ml_dtypes

from concourse import bacc, tile, mybir, masks
from concourse.bass_utils import run_bass_kernel_spmd

BF16 = mybir.dt.bfloat16
F32 = mybir.dt.float32
F8 = mybir.dt.float8e4
AX = mybir.AxisListType
OP = mybir.AluOpType
AF = mybir.ActivationFunctionType

NCORES = 8
NP_ = 2048
NC_ = 64
C_ = 128
D_ = 64
R_ = NP_ // NCORES   # 256
KC = NC_ * C_        # 8192
P_TOPP = 0.9
LN_EPS = 1e-5
N_ITER = 4
GROUPS = [list(range(NCORES))]
SCALE_STAT = 512.0   # fp8 scale on all four stream operators (rfT/M3T/a0T/M4T)
SCALE_W = 8.0        # fp8 scale on projection weights
SCALE_V = 32.0       # fp8 scale on stored V-stream values (512/16)


def _tp128(nc, psum_tp, dst_ap, src_ap, ident, dtype, name):
    """PE transpose of a [128,128] block: src (SBUF) -> dst (SBUF)."""
    ps = psum_tp.tile([128, 128], dtype, name=name, tag="attn_ps")
    nc.tensor.transpose(ps[:], src_ap, ident)
    nc.vector.tensor_copy(dst_ap, ps[:])


def _attention_step(nc, pools, pooledT_loc, pooled_fullT, step):
    """pooledT_loc [128c,256i], pooled_fullT [128c,2048j] (bf16) ->
    two attn tiles [128, 2048] bf16 (row-major, masked + renormalized)."""
    psum_a, small = pools["psum_a"], pools["small"]
    big_lg, big_u = pools["big_lg"], pools["big_u"]
    gT_sb, w3T_sb = pools["gT_sb"], pools["w3T_sb"]

    qT_ps = psum_a.tile([64, R_], F32, name=f"qT_ps{step}", tag="attn_ps")
    nc.tensor.matmul(qT_ps[:], lhsT=gT_sb[:], rhs=pooledT_loc, start=True, stop=True)
    qT_sb = big_lg.tile([64, R_], BF16, name=f"qT_sb{step}", tag="qT_sb")
    nc.scalar.copy(qT_sb[:], qT_ps[:])

    e3T_sb = big_lg.tile([64, NP_], BF16, name=f"e3T_sb{step}", tag="e3T_sb")
    for n in range(4):
        e3_ps = psum_a.tile([64, 512], F32, name=f"e3_ps{step}_{n}", tag="attn_ps")
        nc.tensor.matmul(e3_ps[:], lhsT=w3T_sb[:],
                         rhs=pooled_fullT[:, n * 512:(n + 1) * 512],
                         start=True, stop=True)
        nc.scalar.copy(e3T_sb[:, n * 512:(n + 1) * 512], e3_ps[:])

    ve = nc.vector
    us, scrs, ts, targets, hsums, conds, toffs = [], [], [], [], [], [], []
    for mi in range(2):
        lg = big_lg.tile([128, NP_], F32, name=f"lg{step}_{mi}", tag="logits")
        for n in range(4):
            lg_ps = psum_a.tile([128, 512], F32, name=f"lg_ps{step}_{mi}_{n}",
                                tag="attn_ps")
            nc.tensor.matmul(lg_ps[:], lhsT=qT_sb[:, mi * 128:(mi + 1) * 128],
                             rhs=e3T_sb[:, n * 512:(n + 1) * 512],
                             start=True, stop=True)
            nc.scalar.copy(lg[:, n * 512:(n + 1) * 512], lg_ps[:])

        rmax = small.tile([128, 1], F32, name=f"rmax{step}_{mi}", tag="rmax")
        nc.vector.tensor_reduce(rmax[:], lg[:], axis=AX.X, op=OP.max)
        negmax = small.tile([128, 1], F32, name=f"negmax{step}_{mi}", tag="negmax")
        ve.tensor_scalar_mul(negmax[:], rmax[:], -1.0)
        u = big_u.tile([128, NP_], BF16, name=f"u{step}_{mi}", tag="u")
        zp = small.tile([128, 4], F32, name=f"zp{step}_{mi}", tag="zp")
        for n in range(4):
            nc.scalar.activation(u[:, n * 512:(n + 1) * 512],
                                 lg[:, n * 512:(n + 1) * 512],
                                 AF.Exp, bias=negmax[:], scale=1.0,
                                 accum_out=zp[:, n:n + 1])
        target = small.tile([128, 1], F32, name=f"target{step}_{mi}", tag="target")
        nc.vector.tensor_reduce(target[:], zp[:], axis=AX.X, op=OP.add)
        ve.tensor_scalar_mul(target[:], target[:], P_TOPP)

        t = small.tile([128, 1], F32, name=f"t{step}_{mi}", tag="t")
        ve.memset(t[:], 0.5)
        us.append(u); ts.append(t); targets.append(target)
        scrs.append(big_u.tile([128, NP_], BF16, name=f"scr{step}_{mi}",
                               tag="scratch"))
        hsums.append(small.tile([128, 1], F32, name=f"h{step}_{mi}", tag="hsum"))
        conds.append(small.tile([128, 1], F32, name=f"cond{step}_{mi}",
                                tag="cond"))
        toffs.append(small.tile([128, 1], F32, name=f"toff{step}_{mi}",
                                tag="toff"))

    # interleaved bisection on t (the two row-tiles' serial chains overlap);
    # t_lo is recovered as t_final - 2^-(N_ITER+1) (exact for every branch
    # pattern, including all-fail which telescopes to t_lo = 0).
    for it in range(N_ITER):
        delta = 2.0 ** (-(it + 2))
        for mi in range(2):
            u, t = us[mi], ts[mi]
            ve.scalar_tensor_tensor(scrs[mi][:], u[:], t[:], u[:],
                                    op0=OP.is_gt, op1=OP.mult,
                                    accum_out=hsums[mi][:])
            ve.tensor_scalar_sub(toffs[mi][:], t[:], delta)
            ve.tensor_tensor(conds[mi][:], hsums[mi][:], targets[mi][:],
                             op=OP.is_ge)
            ve.scalar_tensor_tensor(t[:], conds[mi][:], 2.0 * delta,
                                    toffs[mi][:], op0=OP.mult, op1=OP.add)

    for mi in range(2):
        u, t = us[mi], ts[mi]
        ve.tensor_scalar_sub(t[:], t[:], 2.0 ** (-(N_ITER + 1)))
        ssum = small.tile([128, 1], F32, name=f"S{step}_{mi}", tag="ssum")
        ve.scalar_tensor_tensor(scrs[mi][:], u[:], t[:], u[:],
                                op0=OP.is_gt, op1=OP.mult,
                                accum_out=ssum[:])
        rs = small.tile([128, 1], F32, name=f"rS{step}_{mi}", tag="rs")
        nc.vector.reciprocal(rs[:], ssum[:])
        ve.tensor_scalar(u[:], scrs[mi][:], rs[:], None, op0=OP.mult)
    return us


def build_graph(trivial_affine=True, gate=0.5):
    nc = bacc.Bacc("TRN2", target_bir_lowering=False, debug=False,
                   num_devices=NCORES)

    # ---- parameters ----
    # x swizzled on host: xsw[kq*2+jh, p, jb*512+c] = x[jh*1024+jb*128+p,
    # kq*512+c] so every sweep tile load is a dense 4KB/partition DMA.
    xsw = nc.dram_tensor("xsw", [32, 128, 4096], F8, kind="ExternalInput")
    x_loc = nc.dram_tensor("x_loc", [R_, KC], BF16, kind="ExternalInput")
    rfT_f8 = nc.dram_tensor("rfT_f8", [NP_, R_], F8, kind="ExternalInput")
    # rf_sw[p, mb, j] = Rf[mb*128+p, j] * 512 (row-major Rf for M3T lhsT)
    rf_sw = nc.dram_tensor("rf_sw", [128, 16, NP_], F8, kind="ExternalInput")
    gT = nc.dram_tensor("gT", [C_, D_], BF16, kind="ExternalInput")
    w3T = nc.dram_tensor("w3T", [C_, D_], BF16, kind="ExternalInput")
    pooled0T_bf = nc.dram_tensor("pooled0T_bf", [C_, NP_], BF16,
                                 kind="ExternalInput")
    pooled0T_loc_bf = nc.dram_tensor("pooled0T_loc_bf", [C_, R_], BF16,
                                     kind="ExternalInput")
    pooled0_f8 = nc.dram_tensor("pooled0_f8", [NP_, C_], F8,
                                kind="ExternalInput")
    # wpa_f = [wp0T | wp1T] * 8, wpa_a = [wa0T | wa1T] * 8
    wpa_f = nc.dram_tensor("wpa_f", [C_, 2 * C_], F8, kind="ExternalInput")
    wpa_a = nc.dram_tensor("wpa_a", [C_, 2 * C_], F8, kind="ExternalInput")
    if not trivial_affine:
        gamma_rep = nc.dram_tensor("gamma_rep", [128, C_], F32,
                                   kind="ExternalInput")
        beta_rep = nc.dram_tensor("beta_rep", [128, C_], F32,
                                  kind="ExternalInput")
    out_loc = nc.dram_tensor("out_loc", [R_, KC], BF16, kind="ExternalOutput")

    with tile.TileContext(nc) as tc:
        # ---- DRAM bounce buffers ----
        dram = tc.alloc_tile_pool(name="dram", bufs=1, space="DRAM")
        warm_bounce = dram.tile([128, 1], F32, name="warm_bounce")
        warm_full = dram.tile([128 * NCORES, 1], F32, name="warm_full",
                              addr_space="Shared")
        pooled1_bounce = dram.tile([R_, C_], BF16, name="pooled1_bounce")
        pooled1_full = dram.tile([NP_, C_], BF16, name="pooled1_full",
                                 addr_space="Shared")
        a0_bounce = dram.tile([R_, NP_], F8, name="a0_bounce")
        a0_full = dram.tile([NP_, NP_], F8, name="a0_full",
                            addr_space="Shared")

        # ---- persistent SBUF ----
        const_pool = tc.alloc_tile_pool(name="const", bufs=1)
        small_pool = tc.alloc_tile_pool(name="small", bufs=2)

        nc.gpsimd.collective_compute(
            "AllGather", OP.bypass, replica_groups=GROUPS,
            ins=[warm_bounce[:, :]], outs=[warm_full[:, :]])
        ident_bf16 = const_pool.tile([128, 128], BF16, name="ident_bf16")
        masks.make_identity(nc, ident_bf16[:])

        gT_sb = const_pool.tile([C_, D_], BF16, name="gT_sb")
        w3T_sb = const_pool.tile([C_, D_], BF16, name="w3T_sb")
        nc.sync.dma_start(gT_sb[:], gT[:, :])
        nc.sync.dma_start(w3T_sb[:], w3T[:, :])
        pooled0T_sb = const_pool.tile([C_, NP_], BF16, name="pooled0T_sb")
        pooled0T_loc_sb = const_pool.tile([C_, R_], BF16,
                                          name="pooled0T_loc_sb")
        nc.sync.dma_start(pooled0T_loc_sb[:], pooled0T_loc_bf[:, :])
        nc.sync.dma_start(pooled0T_sb[:], pooled0T_bf[:, :])

        # cat4: per jb (16): [0:256)=rfT (M1), [256:512)=M3T,
        # [512:768)=attn0T (M2), [768:1024)=M4T. All fp8 * 512.
        # M1 slots load as ONE strided DMA (16 separate issues would
        # serialize ~10us on the sync queue ahead of rf_sw).
        cat4 = const_pool.tile([128, 16 * 1024], F8, name="cat4")
        nc.sync.dma_start(
            cat4[:].rearrange("p (jb w) -> p jb w", w=1024)[:, :, 0:256],
            rfT_f8[:, :].rearrange("(jb p) i -> p jb i", p=128))
        cat_v = cat4[:].rearrange("p (jbp s w) -> p jbp s w", s=2, w=1024)

        # msrc: rf_sw for M3T, then reused for a0_sw (M4T lhsT)
        # rf_sw on the scalar HWDGE ring: drains in parallel with the sync
        # ring's pooled0T/cat4 loads, so M3T starts ~6us earlier
        msrc_pool = tc.alloc_tile_pool(name="msrc", bufs=1)
        msrc = msrc_pool.tile([128, 16, NP_], F8, name="rf_sw_sb", tag="msrc")
        nc.scalar.dma_start(msrc[:, :, :1024], rf_sw[:, :, :1024])
        nc.scalar.dma_start(msrc[:, :, 1024:], rf_sw[:, :, 1024:])

        p0all = const_pool.tile([128, 16, C_], F8, name="p0all")
        nc.scalar.dma_start(
            p0all[:], pooled0_f8[:, :].rearrange("(jb p) c -> p jb c", p=128))
        wpa_f_sb = const_pool.tile([C_, 2 * C_], F8, name="wpa_f_sb")
        wpa_a_sb = const_pool.tile([C_, 2 * C_], F8, name="wpa_a_sb")
        nc.sync.dma_start(wpa_f_sb[:], wpa_f[:, :])
        nc.sync.dma_start(wpa_a_sb[:], wpa_a[:, :])
        wpa_f_v = wpa_f_sb[:].rearrange("p (s d) -> p s d", s=2)
        wpa_a_v = wpa_a_sb[:].rearrange("p (s d) -> p s d", s=2)
        if not trivial_affine:
            gamma_sb = const_pool.tile([128, C_], F32, name="gamma_sb")
            beta_sb = const_pool.tile([128, C_], F32, name="beta_sb")
            nc.sync.dma_start(gamma_sb[:], gamma_rep[:, :])
            nc.sync.dma_start(beta_sb[:], beta_rep[:, :])

        at1T_sb = const_pool.tile([128, 16 * 256], F8, name="at1T_sb")
        at1_v = at1T_sb[:].rearrange("p (jb s i) -> p jb s i", s=2, i=256)

        resid = [[const_pool.tile([128, 2048], BF16, name=f"resid{mi}_{cc}")
                  for cc in range(4)] for mi in range(2)]
        pooled1_fullT = const_pool.tile([128, NP_], BF16, name="pooled1_fullT")
        pooled1T_loc = const_pool.tile([128, R_], BF16, name="pooled1T_loc")

        psum_a = tc.alloc_tile_pool(name="psum_a", bufs=2, space="PSUM")
        big_lg = tc.alloc_tile_pool(name="big_lg", bufs=2)
        big_u = tc.alloc_tile_pool(name="big_u", bufs=2)

        pools = dict(psum_a=psum_a, small=small_pool, big_lg=big_lg,
                     big_u=big_u, gT_sb=gT_sb, w3T_sb=w3T_sb)

        # ---------- sweep machinery ----------
        psum_b = tc.alloc_tile_pool(name="psum_b", bufs=3, space="PSUM")
        psum_c = tc.alloc_tile_pool(name="psum_c", bufs=3, space="PSUM")
        xs_pool = tc.alloc_tile_pool(name="xstream", bufs=4)
        xx_pool = tc.alloc_tile_pool(name="xx", bufs=2)
        ln_pool = tc.alloc_tile_pool(name="ln_pool", bufs=2)

        def ln_block(col0, w):
            """Fused layernorm + store for cols [col0, col0+w), BOTH mi
            row-tiles merged into [128, 2*w] instructions (halves the
            cross-engine chain steps vs per-mi units)."""
            KWU = (2 * w) // C_
            cs = slice(col0, col0 + w)
            xl2 = ln_pool.tile([128, 2 * w], BF16, name=f"xl2_{col0}",
                               tag="xl2")
            h = ln_pool.tile([128, 2 * w], BF16, name=f"hln_{col0}",
                             tag="hln")
            for mi in range(2):
                ms = slice(mi * w, (mi + 1) * w)
                nc.gpsimd.dma_start(xl2[:, ms],
                                    x_loc[mi * 128:(mi + 1) * 128, cs])
                rsl = resid[mi][col0 // 2048][:, col0 % 2048:
                                              col0 % 2048 + w]
                nc.vector.scalar_tensor_tensor(h[:, ms], rsl, float(gate),
                                               xl2[:, ms],
                                               op0=OP.mult, op1=OP.add)
            hv = h[:].rearrange("p (k c) -> p k c", k=KWU)
            hsq = ln_pool.tile([128, 2 * w], BF16, name=f"hsq_{col0}",
                               tag="hsq")
            nc.scalar.activation(hsq[:], h[:], AF.Square)
            hsqv = hsq[:].rearrange("p (k c) -> p k c", k=KWU)
            s1 = ln_pool.tile([128, KWU, 1], F32, name=f"s1_{col0}", tag="s1")
            s2 = ln_pool.tile([128, KWU, 1], F32, name=f"s2_{col0}", tag="s2")
            nc.vector.tensor_reduce(s1[:], hv, axis=AX.X, op=OP.add)
            nc.vector.tensor_reduce(s2[:], hsqv, axis=AX.X, op=OP.add)
            mu = ln_pool.tile([128, KWU, 1], F32, name=f"mu_{col0}", tag="mu")
            var = ln_pool.tile([128, KWU, 1], F32, name=f"var_{col0}", tag="var")
            rstd = ln_pool.tile([128, KWU, 1], F32, name=f"rstd_{col0}",
                                tag="rstd")
            mb = ln_pool.tile([128, KWU, 1], F32, name=f"mb_{col0}", tag="mb")
            nc.gpsimd.tensor_scalar_mul(mu[:], s1[:], 1.0 / C_)
            nc.gpsimd.tensor_scalar(var[:], s2[:], 1.0 / C_, LN_EPS,
                                    op0=OP.mult, op1=OP.add)
            msq = ln_pool.tile([128, KWU, 1], F32, name=f"msq_{col0}", tag="msq")
            nc.gpsimd.tensor_tensor(msq[:], mu[:], mu[:], op=OP.mult)
            nc.gpsimd.tensor_tensor(var[:], var[:], msq[:], op=OP.subtract)
            sd = ln_pool.tile([128, KWU, 1], F32, name=f"sd_{col0}", tag="sd")
            nc.scalar.activation(sd[:], var[:], AF.Sqrt)
            nc.vector.reciprocal(rstd[:], sd[:])
            nc.gpsimd.tensor_tensor(mb[:], mu[:], rstd[:], op=OP.mult)
            nc.gpsimd.tensor_scalar_mul(mb[:], mb[:], -1.0)
            ov = xl2[:].rearrange("p (k c) -> p k c", k=KWU)
            if (col0 // 1024) % 2 == 0:
                for k in range(KWU):
                    nc.scalar.activation(ov[:, k, :], hv[:, k, :],
                                         AF.Identity, bias=mb[:, k, :],
                                         scale=rstd[:, k, :])
            else:
                rstd_bf = ln_pool.tile([128, KWU, 1], BF16,
                                       name=f"rstdb_{col0}", tag="rstdb")
                mb_bf = ln_pool.tile([128, KWU, 1], BF16,
                                     name=f"mbb_{col0}", tag="mbb")
                nc.gpsimd.tensor_copy(rstd_bf[:], rstd[:])
                nc.gpsimd.tensor_copy(mb_bf[:], mb[:])
                rstd_bc = rstd_bf[:].broadcast_to([128, KWU, C_])
                mb_bc = mb_bf[:].broadcast_to([128, KWU, C_])
                nc.vector.tensor_tensor(ov, hv, rstd_bc, op=OP.mult)
                nc.vector.tensor_tensor(ov, ov, mb_bc, op=OP.add)
            if not trivial_affine:
                g_bc = gamma_sb[:].rearrange(
                    "p (one c) -> p one c", one=1).broadcast_to(
                        [128, KWU, C_])
                b_bc = beta_sb[:].rearrange(
                    "p (one c) -> p one c", one=1).broadcast_to(
                        [128, KWU, C_])
                nc.vector.tensor_tensor(ov, ov, g_bc, op=OP.mult)
                nc.vector.tensor_tensor(ov, ov, b_bc, op=OP.add)
            for mi in range(2):
                ms = slice(mi * w, (mi + 1) * w)
                nc.gpsimd.dma_start(out_loc[mi * 128:(mi + 1) * 128, cs],
                                    xl2[:, ms])

        def sweep_block(kq, pi):
            """pi=0: [rfT|M3T] stream -> resid overwrite (wpa_f).
            pi=1: [a0T|M4T] stream -> resid accumulate (wpa_a) + fused LN."""
            xts = []
            for jh in range(2):
                xt = xs_pool.tile([128, 8, 512], F8,
                                  name=f"xt{pi}_{kq}_{jh}", tag="xt")
                src_ = xsw[kq * 2 + jh, :, :].rearrange(
                    "p (jb c) -> p jb c", c=512)
                nc.sync.dma_start(xt[:, :4, :], src_[:, :4, :])
                nc.sync.dma_start(xt[:, 4:, :], src_[:, 4:, :])
                xts.append(xt)
            xx = xx_pool.tile([128, 4 * 512], F8, name=f"xx{pi}_{kq}",
                              tag="xx")
            xxv = xx[:].rearrange("p (k s w) -> p k s w", s=2, w=256)
            for k4 in range(4):
                ps = psum_b.tile([128, 512], F32, name="psB", tag="psB")
                for jbp in range(8):
                    nc.tensor.matmul(
                        ps[:],
                        lhsT=xts[jbp // 4][:, (jbp % 4) * 2:
                                           (jbp % 4) * 2 + 2,
                                           k4 * 128:(k4 + 1) * 128],
                        rhs=cat_v[:, jbp, :, pi * 512:(pi + 1) * 512],
                        start=(jbp == 0), stop=(jbp == 7),
                        perf_mode=mybir.MatmulPerfMode.DoubleRow)
                if pi == 0 and k4 % 2 == 0:
                    # pass2 keeps DVE free for LN: both copies on scalar
                    nc.vector.tensor_scalar(
                        xx[:, k4 * 512:(k4 + 1) * 512], ps[:],
                        1.0 / 16.0, None, op0=OP.mult)
                else:
                    nc.scalar.mul(xx[:, k4 * 512:(k4 + 1) * 512],
                                  ps[:], 1.0 / 16.0)
            wpa_v = wpa_f_v if pi == 0 else wpa_a_v
            for ig in range(2):
                ps_r = psum_c.tile([128, 512], F32, name="ps_c", tag="ps_c")
                for k4 in range(4):
                    nc.tensor.matmul(
                        ps_r[:, k4 * 128:(k4 + 1) * 128],
                        lhsT=xxv[:, k4, :, ig * 128:(ig + 1) * 128],
                        rhs=wpa_v, start=True, stop=True,
                        perf_mode=mybir.MatmulPerfMode.DoubleRow)
                rsl = resid[ig][kq // 4][:, (kq % 4) * 512:
                                         (kq % 4 + 1) * 512]
                if pi == 0:
                    nc.scalar.mul(rsl, ps_r[:], 1.0 / 256.0)
                else:
                    nc.vector.scalar_tensor_tensor(
                        rsl, ps_r[:], 1.0 / 256.0, rsl,
                        op0=OP.mult, op1=OP.add)
            if pi == 1:
                if kq in (14, 15):
                    # last pair split into halves so the kq14 half
                    # overlaps kq15's matmuls (shorter tail chain)
                    ln_block(kq * 512, 512)
                elif kq % 2 == 1:
                    ln_block((kq // 2) * 1024, 1024)

        def mT_block(dst_off, rhs_w, scale_fac):
            """16 jb x 8 mb-pair MMs: M.T chunks [j128, i256] -> cat4 slots.
            dst_off 256: M3T (rhs = rfT slots); 768: M4T (rhs = at1T)."""
            for jb in range(16):
                ps = psum_a.tile([128, 256], F32, name="psM", tag="attn_ps")
                for mbp in range(8):
                    if dst_off == 256:
                        rhs = cat_v[:, mbp, :, 0:256]
                    else:
                        rhs = at1_v[:, mbp, :, :]
                    nc.tensor.matmul(
                        ps[:],
                        lhsT=msrc[:, 2 * mbp:2 * mbp + 2,
                                  jb * 128:(jb + 1) * 128],
                        rhs=rhs, start=(mbp == 0), stop=(mbp == 7),
                        perf_mode=mybir.MatmulPerfMode.DoubleRow)
                dst = cat4[:, jb * 1024 + dst_off:jb * 1024 + dst_off + 256]
                if jb % 2 == 0:
                    nc.vector.tensor_scalar(dst, ps[:], scale_fac, None,
                                            op0=OP.mult)
                else:
                    nc.scalar.mul(dst, ps[:], scale_fac)

        # ---------------- issue order ----------------
        # attn0 (PE part is small; vector chain runs long)
        attn0 = _attention_step(nc, pools, pooled0T_loc_sb[:],
                                pooled0T_sb[:], 0)

        # M3T: PE work during the attn0 search (no attention dependency)
        mT_block(256, None, 1.0 / 512.0)

        # pass1 head: needs only [M1|M3] slots -- runs during attn0 search
        for kq in range(3):
            sweep_block(kq, 0)

        # attn0 -> cat4 M2 slots. 4 transposes share one psum tile so each
        # copy moves [128, 4x128] in one strided instruction (4x fewer
        # copies -> the post-search transpose burst shortens ~3x).
        cat4_jbv = cat4[:].rearrange("p (jb w) -> p jb w", w=1024)
        for mi in range(2):
            for g in range(4):
                ps = psum_a.tile([128, 4, 128], BF16, name=f"tpa0_{mi}_{g}",
                                 tag="attn_ps")
                for j4 in range(4):
                    jb = g * 4 + j4
                    nc.tensor.transpose(
                        ps[:, j4, :],
                        attn0[mi][:, jb * 128:(jb + 1) * 128], ident_bf16[:])
                dst = cat4_jbv[:, g * 4:(g + 1) * 4,
                               512 + mi * 128:512 + (mi + 1) * 128]
                if mi == 0:
                    nc.vector.tensor_scalar(dst, ps[:], SCALE_STAT, None,
                                            op0=OP.mult)
                else:
                    nc.scalar.mul(dst, ps[:], SCALE_STAT)

        # pooled1 = attn0 @ pooled0 (transposed), allgather. p0all was
        # preloaded in one DMA so the 16-MM chain runs back-to-back.
        ps1 = psum_a.tile([128, R_], F32, name="pooled1_ps", tag="attn_ps")
        for jb in range(16):
            nc.tensor.matmul(ps1[:], lhsT=p0all[:, jb, :],
                             rhs=cat4[:, jb * 1024 + 512:
                                      jb * 1024 + 768],
                             start=(jb == 0), stop=(jb == 15))
        nc.vector.tensor_scalar(pooled1T_loc[:], ps1[:],
                                1.0 / (SCALE_STAT * 8.0),
                                None, op0=OP.mult)

        # pooled1 bounce + AG, then attn0 bounce + AG. The a0st tiles come
        # from the SAME bufs=1 pool/tag as pooled1_rows, so their writes
        # carry a WAR dependency on the pooled1 bounce DMAs -- this forces
        # the CC stream to run the (critical) pooled1 AG before the big a0
        # AG, which Tile's readiness ordering would otherwise invert.
        p1a0_pool = tc.alloc_tile_pool(name="p1a0", bufs=1)
        pooled1_rows = [p1a0_pool.tile([128, NP_], BF16, name=f"pooled1_r{mi}",
                                       tag="p1a0") for mi in range(2)]
        for mi in range(2):
            _tp128(nc, psum_a, pooled1_rows[mi][:, :C_],
                   pooled1T_loc[:, mi * 128:(mi + 1) * 128], ident_bf16[:],
                   BF16, f"tp_p1_{mi}")
            nc.gpsimd.dma_start(pooled1_bounce[mi * 128:(mi + 1) * 128, :],
                                pooled1_rows[mi][:, :C_])
        nc.gpsimd.collective_compute(
            "AllGather", OP.bypass, replica_groups=GROUPS,
            ins=[pooled1_bounce[:, :]], outs=[pooled1_full[:, :]])

        for mi in range(2):
            a0st = p1a0_pool.tile([128, NP_], F8, name=f"a0st{mi}",
                                  tag="p1a0")
            nc.vector.tensor_scalar(a0st[:], attn0[mi][:], SCALE_STAT,
                                    None, op0=OP.mult)
            nc.gpsimd.dma_start(
                a0_bounce[mi * 128:(mi + 1) * 128, :], a0st[:])
        nc.gpsimd.collective_compute(
            "AllGather", OP.bypass, replica_groups=GROUPS,
            ins=[a0_bounce[:, :]], outs=[a0_full[:, :]])

        # pass1 mid (covers the pooled1 AG)
        for kq in range(3, 6):
            sweep_block(kq, 0)

        with tc.tile_pool(name="pf1_pool", bufs=3) as pf1_pool:
            for jb in range(16):
                pt = pf1_pool.tile([128, C_], BF16, name="pf1_t", tag="pf1_t")
                nc.sync.dma_start(pt[:], pooled1_full[jb * 128:(jb + 1) * 128, :])
                _tp128(nc, psum_a,
                       pooled1_fullT[:, jb * 128:(jb + 1) * 128],
                       pt[:], ident_bf16[:], BF16, f"tp_pf1_{jb}")

        attn1 = _attention_step(nc, pools, pooled1T_loc[:], pooled1_fullT[:], 1)

        # pass1 tail (covers the attn1 search)
        for kq in range(6, 11):
            sweep_block(kq, 0)

        # a0_sw: reuse msrc for allgathered attn0 rows (issued mid-pass1
        # so the loads drain well before M4T)
        a0_sw = msrc_pool.tile([128, 16, NP_], F8, name="a0_sw_sb",
                               tag="msrc")
        nc.sync.dma_start(
            a0_sw[:], a0_full[:, :].rearrange("(mb p) j -> p mb j", p=128))
        msrc = a0_sw  # mT_block reads msrc

        for kq in range(11, 16):
            sweep_block(kq, 0)

        # attn1 -> at1T_sb (grouped transposes, as for attn0)
        at1_jbv = at1T_sb[:].rearrange("p (jb w) -> p jb w", w=256)
        for mi in range(2):
            for g in range(4):
                ps = psum_a.tile([128, 4, 128], BF16, name=f"tpa1_{mi}_{g}",
                                 tag="attn_ps")
                for j4 in range(4):
                    jb = g * 4 + j4
                    nc.tensor.transpose(
                        ps[:, j4, :],
                        attn1[mi][:, jb * 128:(jb + 1) * 128], ident_bf16[:])
                dst = at1_jbv[:, g * 4:(g + 1) * 4,
                              mi * 128:(mi + 1) * 128]
                if mi == 0:
                    nc.vector.tensor_scalar(dst, ps[:], SCALE_STAT, None,
                                            op0=OP.mult)
                else:
                    nc.scalar.mul(dst, ps[:], SCALE_STAT)

        # M4T
        mT_block(768, None, 1.0 / 512.0)

        # pass2 + fused LN
        for kq in range(16):
            sweep_block(kq, 1)

        p1a0_pool.release()
        ln_pool.release()
        xx_pool.release()
        xs_pool.release()
        psum_c.release()
        psum_b.release()
        big_u.release()
        big_lg.release()
        psum_a.release()
        msrc_pool.release()
        small_pool.release()
        const_pool.release()
        dram.release()

    nc.finalize()
    return nc


# ---------------------------------------------------------------------------
# Host side
# ---------------------------------------------------------------------------
_CACHE = {}


def _get_graph(trivial_affine, gate):
    key = (bool(trivial_affine), round(float(gate), 8))
    if key not in _CACHE:
        _CACHE[key] = build_graph(key[0], gate=key[1])
    return _CACHE[key]


def prepare_in_maps(x, prior, W1, W2, W3, prior_fwd_w, adaptive_w,
                    ln_gamma, ln_beta, alpha):
    bf = ml_dtypes.bfloat16
    f8 = ml_dtypes.float8_e4m3
    x2 = np.ascontiguousarray(np.asarray(x, np.float32).reshape(NP_, KC))
    x_f8 = x2.astype(f8)
    # xsw[kq*2+jh, p, jb*512+c] = x_f8[jh*1024+jb*128+p, kq*512+c]
    xsw = np.ascontiguousarray(
        x_f8.reshape(2, 8, 128, 16, 512).transpose(3, 0, 2, 1, 4).reshape(
            32, 128, 4096))
    pooled0 = np.asarray(x, np.float32).reshape(NP_, NC_, C_).mean(axis=1)
    pooled0T_bf = np.ascontiguousarray(pooled0.T).astype(bf)     # [C, NP]
    pooled0_f8 = (pooled0 * 8.0).astype(f8)                      # [NP, C]
    prior = np.asarray(prior, np.float32)
    rs = np.maximum(prior.sum(axis=1, keepdims=True), 1e-12)
    rf = (prior / rs).astype(np.float32)
    rf512 = (rf * 512.0).astype(f8)
    # rf_sw[p, mb, j] = (512*Rf)[mb*128+p, j]
    rf_sw = np.ascontiguousarray(
        rf512.reshape(16, 128, NP_).transpose(1, 0, 2))

    W1 = np.asarray(W1, np.float32)
    W2 = np.asarray(W2, np.float32)
    W3 = np.asarray(W3, np.float32)
    G = (W2 @ W1)                       # [D, C]
    gT_h = np.ascontiguousarray(G.T).astype(bf)       # [C, D]
    w3T_h = np.ascontiguousarray(W3.T).astype(bf)     # [C, D]

    pw = np.asarray(prior_fwd_w, np.float32)
    aw = np.asarray(adaptive_w, np.float32)
    wp0T = (np.ascontiguousarray(pw[0].T) * SCALE_W).astype(f8)
    wp1T = (np.ascontiguousarray(pw[1].T) * SCALE_W).astype(f8)
    wa0T = (np.ascontiguousarray(aw[0].T) * SCALE_W).astype(f8)
    wa1T = (np.ascontiguousarray(aw[1].T) * SCALE_W).astype(f8)
    wpa_f = np.ascontiguousarray(np.concatenate([wp0T, wp1T], axis=1))
    wpa_a = np.ascontiguousarray(np.concatenate([wa0T, wa1T], axis=1))

    gate = 1.0 / (1.0 + np.exp(-np.float32(np.asarray(alpha).reshape(-1)[0])))

    gamma = np.asarray(ln_gamma, np.float32)
    beta = np.asarray(ln_beta, np.float32)
    trivial_affine = bool(np.all(gamma == 1.0) and np.all(beta == 0.0))

    in_maps = []
    for c in range(NCORES):
        rows = slice(c * R_, (c + 1) * R_)
        m = {
            "xsw": xsw,
            "x_loc": x2[rows].astype(bf),
            "rfT_f8": np.ascontiguousarray(rf512[rows].T),
            "rf_sw": rf_sw,
            "gT": gT_h,
            "w3T": w3T_h,
            "pooled0T_bf": pooled0T_bf,
            "pooled0T_loc_bf": np.ascontiguousarray(pooled0T_bf[:, rows]),
            "pooled0_f8": pooled0_f8,
            "wpa_f": wpa_f, "wpa_a": wpa_a,
        }
        if not trivial_affine:
            m["gamma_rep"] = np.broadcast_to(gamma, (128, C_)).copy()
            m["beta_rep"] = np.broadcast_to(beta, (128, C_)).copy()
        in_maps.append(m)
    return in_maps, trivial_affine, gate


def run(x, prior, W1, W2, W3, prior_fwd_w, adaptive_w, ln_gamma, ln_beta,
        alpha, trace=False):
    in_maps, trivial_affine, gate = prepare_in_maps(
        x, prior, W1, W2, W3, prior_fwd_w, adaptive_w, ln_gamma, ln_beta, alpha)
    nc = _get_graph(trivial_affine, gate)
    res = run_bass_kernel_spmd(nc, in_maps, core_ids=list(range(NCORES)),
                               trace=trace)
    out = np.concatenate([np.asarray(res.results[c]["out_loc"]).astype(
        np.float32) for c in range(NCORES)], axis=0)
    return out.reshape(NP_, NC_, C_), res


def kernel(x, prior, W1, W2, W3, prior_fwd_w, adaptive_w, ln_gamma, ln_beta,
           alpha):
    out, _ = run(x, prior, W1, W2, W3, prior_fwd_w, adaptive_w, ln_gamma,
                 ln_beta, alpha, trace=False)
    return out


# revision 44
# speedup vs baseline: 1.1931x; 1.1931x over previous
"""Trainium2 Bass kernel for AdaptiveDiffusionBlock (8 NeuronCores, SPMD).

Row-shards N_P=2048 over 8 cores (256 rows each). v2 restructure:

    residual = Xf1@Wp0.T + Xf2@Wp1.T + Xa1@Wa0.T + Xa2@Wa1.T
    Xf1 = Rf@x, Xf2 = (Rf@Rf)@x = M3@x, Xa1 = attn0@x, Xa2 = (attn1@attn0)@x = M4@x

Two stationary-x sweeps: pass1 streams [rfT | M3T] (no attention dep),
pass2 streams [attn0T | M4T] with LayerNorm fused per column chunk.
M3T computed on-device from host-supplied row-major Rf (rf_sw); M4T from
an attn0 AllGather (4 MiB fp8 -- the only large collective; the v1 P/Q
allgathers, projections and stage-E are gone). Top-p thresholds via
4-iteration binary search on t in (0,1] (u = exp(l - rowmax)).

kernel(**inputs) takes full numpy inputs, returns the full output.
"""

import sys

for _p in ("/opt/trn_rl_repo", "/root/.axon_site", "/root/.axon_site/_ro/trn_rl_repo"):
    if _p not in sys.path:
        sys.path.append(_p)

import numpy as np
import ml_dtypes

from concourse import bacc, tile, mybir, masks
from concourse.bass_utils import run_bass_kernel_spmd

BF16 = mybir.dt.bfloat16
F32 = mybir.dt.float32
F8 = mybir.dt.float8e4
AX = mybir.AxisListType
OP = mybir.AluOpType
AF = mybir.ActivationFunctionType

NCORES = 8
NP_ = 2048
NC_ = 64
C_ = 128
D_ = 64
R_ = NP_ // NCORES   # 256
KC = NC_ * C_        # 8192
P_TOPP = 0.9
LN_EPS = 1e-5
N_ITER = 4
GROUPS = [list(range(NCORES))]
SCALE_STAT = 512.0   # fp8 scale on all four stream operators (rfT/M3T/a0T/M4T)
SCALE_W = 8.0        # fp8 scale on projection weights
SCALE_V = 32.0       # fp8 scale on stored V-stream values (512/16)


def _tp128(nc, psum_tp, dst_ap, src_ap, ident, dtype, name):
    """PE transpose of a [128,128] block: src (SBUF) -> dst (SBUF)."""
    ps = psum_tp.tile([128, 128], dtype, name=name, tag="attn_ps")
    nc.tensor.transpose(ps[:], src_ap, ident)
    nc.vector.tensor_copy(dst_ap, ps[:])


def _attention_step(nc, pools, pooledT_loc, pooled_fullT, step):
    """pooledT_loc [128c,256i], pooled_fullT [128c,2048j] (bf16) ->
    two attn tiles [128, 2048] bf16 (row-major, masked + renormalized)."""
    psum_a, small = pools["psum_a"], pools["small"]
    big_lg, big_u = pools["big_lg"], pools["big_u"]
    gT_sb, w3T_sb = pools["gT_sb"], pools["w3T_sb"]

    qT_ps = psum_a.tile([64, R_], F32, name=f"qT_ps{step}", tag="attn_ps")
    nc.tensor.matmul(qT_ps[:], lhsT=gT_sb[:], rhs=pooledT_loc, start=True, stop=True)
    qT_sb = big_lg.tile([64, R_], BF16, name=f"qT_sb{step}", tag="qT_sb")
    nc.scalar.copy(qT_sb[:], qT_ps[:])

    e3T_sb = big_lg.tile([64, NP_], BF16, name=f"e3T_sb{step}", tag="e3T_sb")
    for n in range(4):
        e3_ps = psum_a.tile([64, 512], F32, name=f"e3_ps{step}_{n}", tag="attn_ps")
        nc.tensor.matmul(e3_ps[:], lhsT=w3T_sb[:],
                         rhs=pooled_fullT[:, n * 512:(n + 1) * 512],
                         start=True, stop=True)
        nc.scalar.copy(e3T_sb[:, n * 512:(n + 1) * 512], e3_ps[:])

    ve = nc.vector
    us, scrs, ts, targets, hsums, conds, toffs = [], [], [], [], [], [], []
    for mi in range(2):
        lg = big_lg.tile([128, NP_], F32, name=f"lg{step}_{mi}", tag="logits")
        for n in range(4):
            lg_ps = psum_a.tile([128, 512], F32, name=f"lg_ps{step}_{mi}_{n}",
                                tag="attn_ps")
            nc.tensor.matmul(lg_ps[:], lhsT=qT_sb[:, mi * 128:(mi + 1) * 128],
                             rhs=e3T_sb[:, n * 512:(n + 1) * 512],
                             start=True, stop=True)
            nc.scalar.copy(lg[:, n * 512:(n + 1) * 512], lg_ps[:])

        rmax = small.tile([128, 1], F32, name=f"rmax{step}_{mi}", tag="rmax")
        nc.vector.tensor_reduce(rmax[:], lg[:], axis=AX.X, op=OP.max)
        negmax = small.tile([128, 1], F32, name=f"negmax{step}_{mi}", tag="negmax")
        ve.tensor_scalar_mul(negmax[:], rmax[:], -1.0)
        u = big_u.tile([128, NP_], BF16, name=f"u{step}_{mi}", tag="u")
        zp = small.tile([128, 4], F32, name=f"zp{step}_{mi}", tag="zp")
        for n in range(4):
            nc.scalar.activation(u[:, n * 512:(n + 1) * 512],
                                 lg[:, n * 512:(n + 1) * 512],
                                 AF.Exp, bias=negmax[:], scale=1.0,
                                 accum_out=zp[:, n:n + 1])
        target = small.tile([128, 1], F32, name=f"target{step}_{mi}", tag="target")
        nc.vector.tensor_reduce(target[:], zp[:], axis=AX.X, op=OP.add)
        ve.tensor_scalar_mul(target[:], target[:], P_TOPP)

        t = small.tile([128, 1], F32, name=f"t{step}_{mi}", tag="t")
        ve.memset(t[:], 0.5)
        us.append(u); ts.append(t); targets.append(target)
        scrs.append(big_u.tile([128, NP_], BF16, name=f"scr{step}_{mi}",
                               tag="scratch"))
        hsums.append(small.tile([128, 1], F32, name=f"h{step}_{mi}", tag="hsum"))
        conds.append(small.tile([128, 1], F32, name=f"cond{step}_{mi}",
                                tag="cond"))
        toffs.append(small.tile([128, 1], F32, name=f"toff{step}_{mi}",
                                tag="toff"))

    # interleaved bisection on t (the two row-tiles' serial chains overlap);
    # t_lo is recovered as t_final - 2^-(N_ITER+1) (exact for every branch
    # pattern, including all-fail which telescopes to t_lo = 0).
    for it in range(N_ITER):
        delta = 2.0 ** (-(it + 2))
        for mi in range(2):
            u, t = us[mi], ts[mi]
            ve.scalar_tensor_tensor(scrs[mi][:], u[:], t[:], u[:],
                                    op0=OP.is_gt, op1=OP.mult,
                                    accum_out=hsums[mi][:])
            ve.tensor_scalar_sub(toffs[mi][:], t[:], delta)
            ve.tensor_tensor(conds[mi][:], hsums[mi][:], targets[mi][:],
                             op=OP.is_ge)
            ve.scalar_tensor_tensor(t[:], conds[mi][:], 2.0 * delta,
                                    toffs[mi][:], op0=OP.mult, op1=OP.add)

    for mi in range(2):
        u, t = us[mi], ts[mi]
        ve.tensor_scalar_sub(t[:], t[:], 2.0 ** (-(N_ITER + 1)))
        ssum = small.tile([128, 1], F32, name=f"S{step}_{mi}", tag="ssum")
        ve.scalar_tensor_tensor(scrs[mi][:], u[:], t[:], u[:],
                                op0=OP.is_gt, op1=OP.mult,
                                accum_out=ssum[:])
        rs = small.tile([128, 1], F32, name=f"rS{step}_{mi}", tag="rs")
        nc.vector.reciprocal(rs[:], ssum[:])
        ve.tensor_scalar(u[:], scrs[mi][:], rs[:], None, op0=OP.mult)
    return us


def build_graph(trivial_affine=True, gate=0.5):
    nc = bacc.Bacc("TRN2", target_bir_lowering=False, debug=False,
                   num_devices=NCORES)

    # ---- parameters ----
    # x swizzled on host: xsw[kq*2+jh, p, jb*512+c] = x[jh*1024+jb*128+p,
    # kq*512+c] so every sweep tile load is a dense 4KB/partition DMA.
    xsw = nc.dram_tensor("xsw", [32, 128, 4096], F8, kind="ExternalInput")
    x_loc = nc.dram_tensor("x_loc", [R_, KC], BF16, kind="ExternalInput")
    rfT_f8 = nc.dram_tensor("rfT_f8", [NP_, R_], F8, kind="ExternalInput")
    # rf_sw[p, mb, j] = Rf[mb*128+p, j] * 512 (row-major Rf for M3T lhsT)
    rf_sw = nc.dram_tensor("rf_sw", [128, 16, NP_], F8, kind="ExternalInput")
    gT = nc.dram_tensor("gT", [C_, D_], BF16, kind="ExternalInput")
    w3T = nc.dram_tensor("w3T", [C_, D_], BF16, kind="ExternalInput")
    pooled0T_bf = nc.dram_tensor("pooled0T_bf", [C_, NP_], BF16,
                                 kind="ExternalInput")
    pooled0T_loc_bf = nc.dram_tensor("pooled0T_loc_bf", [C_, R_], BF16,
                                     kind="ExternalInput")
    pooled0_f8 = nc.dram_tensor("pooled0_f8", [NP_, C_], F8,
                                kind="ExternalInput")
    # wpa_f = [wp0T | wp1T] * 8, wpa_a = [wa0T | wa1T] * 8
    wpa_f = nc.dram_tensor("wpa_f", [C_, 2 * C_], F8, kind="ExternalInput")
    wpa_a = nc.dram_tensor("wpa_a", [C_, 2 * C_], F8, kind="ExternalInput")
    if not trivial_affine:
        gamma_rep = nc.dram_tensor("gamma_rep", [128, C_], F32,
                                   kind="ExternalInput")
        beta_rep = nc.dram_tensor("beta_rep", [128, C_], F32,
                                  kind="ExternalInput")
    out_loc = nc.dram_tensor("out_loc", [R_, KC], BF16, kind="ExternalOutput")

    with tile.TileContext(nc) as tc:
        # ---- DRAM bounce buffers ----
        dram = tc.alloc_tile_pool(name="dram", bufs=1, space="DRAM")
        warm_bounce = dram.tile([128, 1], F32, name="warm_bounce")
        warm_full = dram.tile([128 * NCORES, 1], F32, name="warm_full",
                              addr_space="Shared")
        pooled1_bounce = dram.tile([R_, C_], BF16, name="pooled1_bounce")
        pooled1_full = dram.tile([NP_, C_], BF16, name="pooled1_full",
                                 addr_space="Shared")
        a0_bounce = dram.tile([R_, NP_], F8, name="a0_bounce")
        a0_full = dram.tile([NP_, NP_], F8, name="a0_full",
                            addr_space="Shared")

        # ---- persistent SBUF ----
        const_pool = tc.alloc_tile_pool(name="const", bufs=1)
        small_pool = tc.alloc_tile_pool(name="small", bufs=2)

        nc.gpsimd.collective_compute(
            "AllGather", OP.bypass, replica_groups=GROUPS,
            ins=[warm_bounce[:, :]], outs=[warm_full[:, :]])
        ident_bf16 = const_pool.tile([128, 128], BF16, name="ident_bf16")
        masks.make_identity(nc, ident_bf16[:])

        gT_sb = const_pool.tile([C_, D_], BF16, name="gT_sb")
        w3T_sb = const_pool.tile([C_, D_], BF16, name="w3T_sb")
        nc.sync.dma_start(gT_sb[:], gT[:, :])
        nc.sync.dma_start(w3T_sb[:], w3T[:, :])
        pooled0T_sb = const_pool.tile([C_, NP_], BF16, name="pooled0T_sb")
        pooled0T_loc_sb = const_pool.tile([C_, R_], BF16,
                                          name="pooled0T_loc_sb")
        nc.sync.dma_start(pooled0T_loc_sb[:], pooled0T_loc_bf[:, :])
        nc.sync.dma_start(pooled0T_sb[:], pooled0T_bf[:, :])

        # cat4: per jb (16): [0:256)=rfT (M1), [256:512)=M3T,
        # [512:768)=attn0T (M2), [768:1024)=M4T. All fp8 * 512.
        # M1 slots load as ONE strided DMA (16 separate issues would
        # serialize ~10us on the sync queue ahead of rf_sw).
        cat4 = const_pool.tile([128, 16 * 1024], F8, name="cat4")
        nc.sync.dma_start(
            cat4[:].rearrange("p (jb w) -> p jb w", w=1024)[:, :, 0:256],
            rfT_f8[:, :].rearrange("(jb p) i -> p jb i", p=128))
        cat_v = cat4[:].rearrange("p (jbp s w) -> p jbp s w", s=2, w=1024)

        # msrc: rf_sw for M3T, then reused for a0_sw (M4T lhsT)
        # rf_sw on the scalar HWDGE ring: drains in parallel with the sync
        # ring's pooled0T/cat4 loads, so M3T starts ~6us earlier
        msrc_pool = tc.alloc_tile_pool(name="msrc", bufs=1)
        msrc = msrc_pool.tile([128, 16, NP_], F8, name="rf_sw_sb", tag="msrc")
        nc.sync.dma_start(msrc[:, :, :1024], rf_sw[:, :, :1024])
        nc.sync.dma_start(msrc[:, :, 1024:], rf_sw[:, :, 1024:])

        p0all = const_pool.tile([128, 16, C_], F8, name="p0all")
        nc.sync.dma_start(
            p0all[:], pooled0_f8[:, :].rearrange("(jb p) c -> p jb c", p=128))
        wpa_f_sb = const_pool.tile([C_, 2 * C_], F8, name="wpa_f_sb")
        wpa_a_sb = const_pool.tile([C_, 2 * C_], F8, name="wpa_a_sb")
        nc.sync.dma_start(wpa_f_sb[:], wpa_f[:, :])
        nc.sync.dma_start(wpa_a_sb[:], wpa_a[:, :])
        wpa_f_v = wpa_f_sb[:].rearrange("p (s d) -> p s d", s=2)
        wpa_a_v = wpa_a_sb[:].rearrange("p (s d) -> p s d", s=2)
        if not trivial_affine:
            gamma_sb = const_pool.tile([128, C_], F32, name="gamma_sb")
            beta_sb = const_pool.tile([128, C_], F32, name="beta_sb")
            nc.sync.dma_start(gamma_sb[:], gamma_rep[:, :])
            nc.sync.dma_start(beta_sb[:], beta_rep[:, :])

        at1T_sb = const_pool.tile([128, 16 * 256], F8, name="at1T_sb")
        at1_v = at1T_sb[:].rearrange("p (jb s i) -> p jb s i", s=2, i=256)

        resid = [[const_pool.tile([128, 2048], BF16, name=f"resid{mi}_{cc}")
                  for cc in range(4)] for mi in range(2)]
        pooled1_fullT = const_pool.tile([128, NP_], BF16, name="pooled1_fullT")
        pooled1T_loc = const_pool.tile([128, R_], BF16, name="pooled1T_loc")

        psum_a = tc.alloc_tile_pool(name="psum_a", bufs=2, space="PSUM")
        big_lg = tc.alloc_tile_pool(name="big_lg", bufs=2)
        big_u = tc.alloc_tile_pool(name="big_u", bufs=2)

        pools = dict(psum_a=psum_a, small=small_pool, big_lg=big_lg,
                     big_u=big_u, gT_sb=gT_sb, w3T_sb=w3T_sb)

        # ---------- sweep machinery ----------
        psum_b = tc.alloc_tile_pool(name="psum_b", bufs=3, space="PSUM")
        psum_c = tc.alloc_tile_pool(name="psum_c", bufs=3, space="PSUM")
        xs_pool = tc.alloc_tile_pool(name="xstream", bufs=4)
        xx_pool = tc.alloc_tile_pool(name="xx", bufs=2)
        ln_pool = tc.alloc_tile_pool(name="ln_pool", bufs=2)

        def ln_block(col0, w):
            """Fused layernorm + store for cols [col0, col0+w), BOTH mi
            row-tiles merged into [128, 2*w] instructions (halves the
            cross-engine chain steps vs per-mi units)."""
            KWU = (2 * w) // C_
            cs = slice(col0, col0 + w)
            xl2 = ln_pool.tile([128, 2 * w], BF16, name=f"xl2_{col0}",
                               tag="xl2")
            h = ln_pool.tile([128, 2 * w], BF16, name=f"hln_{col0}",
                             tag="hln")
            for mi in range(2):
                ms = slice(mi * w, (mi + 1) * w)
                nc.gpsimd.dma_start(xl2[:, ms],
                                    x_loc[mi * 128:(mi + 1) * 128, cs])
                rsl = resid[mi][col0 // 2048][:, col0 % 2048:
                                              col0 % 2048 + w]
                nc.vector.scalar_tensor_tensor(h[:, ms], rsl, float(gate),
                                               xl2[:, ms],
                                               op0=OP.mult, op1=OP.add)
            hv = h[:].rearrange("p (k c) -> p k c", k=KWU)
            hsq = ln_pool.tile([128, 2 * w], BF16, name=f"hsq_{col0}",
                               tag="hsq")
            nc.scalar.activation(hsq[:], h[:], AF.Square)
            hsqv = hsq[:].rearrange("p (k c) -> p k c", k=KWU)
            s1 = ln_pool.tile([128, KWU, 1], F32, name=f"s1_{col0}", tag="s1")
            s2 = ln_pool.tile([128, KWU, 1], F32, name=f"s2_{col0}", tag="s2")
            nc.vector.tensor_reduce(s1[:], hv, axis=AX.X, op=OP.add)
            nc.vector.tensor_reduce(s2[:], hsqv, axis=AX.X, op=OP.add)
            mu = ln_pool.tile([128, KWU, 1], F32, name=f"mu_{col0}", tag="mu")
            var = ln_pool.tile([128, KWU, 1], F32, name=f"var_{col0}", tag="var")
            rstd = ln_pool.tile([128, KWU, 1], F32, name=f"rstd_{col0}",
                                tag="rstd")
            mb = ln_pool.tile([128, KWU, 1], F32, name=f"mb_{col0}", tag="mb")
            nc.gpsimd.tensor_scalar_mul(mu[:], s1[:], 1.0 / C_)
            nc.gpsimd.tensor_scalar(var[:], s2[:], 1.0 / C_, LN_EPS,
                                    op0=OP.mult, op1=OP.add)
            msq = ln_pool.tile([128, KWU, 1], F32, name=f"msq_{col0}", tag="msq")
            nc.gpsimd.tensor_tensor(msq[:], mu[:], mu[:], op=OP.mult)
            nc.gpsimd.tensor_tensor(var[:], var[:], msq[:], op=OP.subtract)
            sd = ln_pool.tile([128, KWU, 1], F32, name=f"sd_{col0}", tag="sd")
            nc.scalar.activation(sd[:], var[:], AF.Sqrt)
            nc.vector.reciprocal(rstd[:], sd[:])
            nc.gpsimd.tensor_tensor(mb[:], mu[:], rstd[:], op=OP.mult)
            nc.gpsimd.tensor_scalar_mul(mb[:], mb[:], -1.0)
            ov = xl2[:].rearrange("p (k c) -> p k c", k=KWU)
            # normalize split between scalar (first half k-groups, identity
            # with per-group bias/scale) and vector (broadcast mul+add) so
            # neither engine becomes the pass2 bottleneck
            half = KWU // 2
            for k in range(half):
                nc.scalar.activation(ov[:, k, :], hv[:, k, :],
                                     AF.Identity, bias=mb[:, k, :],
                                     scale=rstd[:, k, :])
            rstd_bf = ln_pool.tile([128, KWU, 1], BF16,
                                   name=f"rstdb_{col0}", tag="rstdb")
            mb_bf = ln_pool.tile([128, KWU, 1], BF16,
                                 name=f"mbb_{col0}", tag="mbb")
            nc.gpsimd.tensor_copy(rstd_bf[:], rstd[:])
            nc.gpsimd.tensor_copy(mb_bf[:], mb[:])
            nh = KWU - half
            rstd_bc = rstd_bf[:, half:, :].broadcast_to([128, nh, C_])
            mb_bc = mb_bf[:, half:, :].broadcast_to([128, nh, C_])
            nc.vector.tensor_tensor(ov[:, half:, :], hv[:, half:, :],
                                    rstd_bc, op=OP.mult)
            nc.vector.tensor_tensor(ov[:, half:, :], ov[:, half:, :],
                                    mb_bc, op=OP.add)
            if not trivial_affine:
                g_bc = gamma_sb[:].rearrange(
                    "p (one c) -> p one c", one=1).broadcast_to(
                        [128, KWU, C_])
                b_bc = beta_sb[:].rearrange(
                    "p (one c) -> p one c", one=1).broadcast_to(
                        [128, KWU, C_])
                nc.vector.tensor_tensor(ov, ov, g_bc, op=OP.mult)
                nc.vector.tensor_tensor(ov, ov, b_bc, op=OP.add)
            for mi in range(2):
                ms = slice(mi * w, (mi + 1) * w)
                nc.gpsimd.dma_start(out_loc[mi * 128:(mi + 1) * 128, cs],
                                    xl2[:, ms])

        def sweep_block(kq, pi):
            """pi=0: [rfT|M3T] stream -> resid overwrite (wpa_f).
            pi=1: [a0T|M4T] stream -> resid accumulate (wpa_a) + fused LN."""
            xts = []
            for jh in range(2):
                xt = xs_pool.tile([128, 8, 512], F8,
                                  name=f"xt{pi}_{kq}_{jh}", tag="xt")
                src_ = xsw[kq * 2 + jh, :, :].rearrange(
                    "p (jb c) -> p jb c", c=512)
                nc.sync.dma_start(xt[:, :4, :], src_[:, :4, :])
                nc.sync.dma_start(xt[:, 4:, :], src_[:, 4:, :])
                xts.append(xt)
            xx = xx_pool.tile([128, 4 * 512], F8, name=f"xx{pi}_{kq}",
                              tag="xx")
            xxv = xx[:].rearrange("p (k s w) -> p k s w", s=2, w=256)
            for k4 in range(4):
                ps = psum_b.tile([128, 512], F32, name="psB", tag="psB")
                for jbp in range(8):
                    nc.tensor.matmul(
                        ps[:],
                        lhsT=xts[jbp // 4][:, (jbp % 4) * 2:
                                           (jbp % 4) * 2 + 2,
                                           k4 * 128:(k4 + 1) * 128],
                        rhs=cat_v[:, jbp, :, pi * 512:(pi + 1) * 512],
                        start=(jbp == 0), stop=(jbp == 7),
                        perf_mode=mybir.MatmulPerfMode.DoubleRow)
                # all sweep copies on scalar: DVE stays free for the
                # attention search / transpose copies (pass1) and LN (pass2)
                nc.scalar.mul(xx[:, k4 * 512:(k4 + 1) * 512],
                              ps[:], 1.0 / 16.0)
            wpa_v = wpa_f_v if pi == 0 else wpa_a_v
            for ig in range(2):
                ps_r = psum_c.tile([128, 512], F32, name="ps_c", tag="ps_c")
                for k4 in range(4):
                    nc.tensor.matmul(
                        ps_r[:, k4 * 128:(k4 + 1) * 128],
                        lhsT=xxv[:, k4, :, ig * 128:(ig + 1) * 128],
                        rhs=wpa_v, start=True, stop=True,
                        perf_mode=mybir.MatmulPerfMode.DoubleRow)
                rsl = resid[ig][kq // 4][:, (kq % 4) * 512:
                                         (kq % 4 + 1) * 512]
                if pi == 0:
                    nc.scalar.mul(rsl, ps_r[:], 1.0 / 256.0)
                else:
                    nc.vector.scalar_tensor_tensor(
                        rsl, ps_r[:], 1.0 / 256.0, rsl,
                        op0=OP.mult, op1=OP.add)
            if pi == 1:
                if kq in (14, 15):
                    # last pair split into halves so the kq14 half
                    # overlaps kq15's matmuls (shorter tail chain)
                    ln_block(kq * 512, 512)
                elif kq % 2 == 1:
                    ln_block((kq // 2) * 1024, 1024)

        def mT_block(dst_off, rhs_w, scale_fac):
            """16 jb x 8 mb-pair MMs: M.T chunks [j128, i256] -> cat4 slots.
            dst_off 256: M3T (rhs = rfT slots); 768: M4T (rhs = at1T)."""
            for jb in range(16):
                ps = psum_a.tile([128, 256], F32, name="psM", tag="attn_ps")
                for mbp in range(8):
                    if dst_off == 256:
                        rhs = cat_v[:, mbp, :, 0:256]
                    else:
                        rhs = at1_v[:, mbp, :, :]
                    nc.tensor.matmul(
                        ps[:],
                        lhsT=msrc[:, 2 * mbp:2 * mbp + 2,
                                  jb * 128:(jb + 1) * 128],
                        rhs=rhs, start=(mbp == 0), stop=(mbp == 7),
                        perf_mode=mybir.MatmulPerfMode.DoubleRow)
                dst = cat4[:, jb * 1024 + dst_off:jb * 1024 + dst_off + 256]
                if jb % 2 == 0:
                    nc.vector.tensor_scalar(dst, ps[:], scale_fac, None,
                                            op0=OP.mult)
                else:
                    nc.scalar.mul(dst, ps[:], scale_fac)

        # ---------------- issue order ----------------
        # attn0 (PE part is small; vector chain runs long)
        attn0 = _attention_step(nc, pools, pooled0T_loc_sb[:],
                                pooled0T_sb[:], 0)

        # M3T: PE work during the attn0 search (no attention dependency)
        mT_block(256, None, 1.0 / 512.0)

        # pass1 head: needs only [M1|M3] slots -- runs during attn0 search
        for kq in range(3):
            sweep_block(kq, 0)

        # attn0 -> cat4 M2 slots. 4 transposes share one psum tile so each
        # copy moves [128, 4x128] in one strided instruction (4x fewer
        # copies -> the post-search transpose burst shortens ~3x).
        cat4_jbv = cat4[:].rearrange("p (jb w) -> p jb w", w=1024)
        for mi in range(2):
            for g in range(4):
                ps = psum_a.tile([128, 4, 128], BF16, name=f"tpa0_{mi}_{g}",
                                 tag="attn_ps")
                for j4 in range(4):
                    jb = g * 4 + j4
                    nc.tensor.transpose(
                        ps[:, j4, :],
                        attn0[mi][:, jb * 128:(jb + 1) * 128], ident_bf16[:])
                dst = cat4_jbv[:, g * 4:(g + 1) * 4,
                               512 + mi * 128:512 + (mi + 1) * 128]
                nc.vector.tensor_scalar(dst, ps[:], SCALE_STAT, None,
                                        op0=OP.mult)

        # pooled1 = attn0 @ pooled0 (transposed), allgather. p0all was
        # preloaded in one DMA so the 16-MM chain runs back-to-back.
        ps1 = psum_a.tile([128, R_], F32, name="pooled1_ps", tag="attn_ps")
        for jb in range(16):
            nc.tensor.matmul(ps1[:], lhsT=p0all[:, jb, :],
                             rhs=cat4[:, jb * 1024 + 512:
                                      jb * 1024 + 768],
                             start=(jb == 0), stop=(jb == 15))
        nc.vector.tensor_scalar(pooled1T_loc[:], ps1[:],
                                1.0 / (SCALE_STAT * 8.0),
                                None, op0=OP.mult)

        # pooled1 bounce + AG, then attn0 bounce + AG. The a0st tiles come
        # from the SAME bufs=1 pool/tag as pooled1_rows, so their writes
        # carry a WAR dependency on the pooled1 bounce DMAs -- this forces
        # the CC stream to run the (critical) pooled1 AG before the big a0
        # AG, which Tile's readiness ordering would otherwise invert.
        p1a0_pool = tc.alloc_tile_pool(name="p1a0", bufs=1)
        pooled1_rows = [p1a0_pool.tile([128, NP_], BF16, name=f"pooled1_r{mi}",
                                       tag="p1a0") for mi in range(2)]
        for mi in range(2):
            _tp128(nc, psum_a, pooled1_rows[mi][:, :C_],
                   pooled1T_loc[:, mi * 128:(mi + 1) * 128], ident_bf16[:],
                   BF16, f"tp_p1_{mi}")
            nc.gpsimd.dma_start(pooled1_bounce[mi * 128:(mi + 1) * 128, :],
                                pooled1_rows[mi][:, :C_])
        nc.gpsimd.collective_compute(
            "AllGather", OP.bypass, replica_groups=GROUPS,
            ins=[pooled1_bounce[:, :]], outs=[pooled1_full[:, :]])

        for mi in range(2):
            a0st = p1a0_pool.tile([128, NP_], F8, name=f"a0st{mi}",
                                  tag="p1a0")
            nc.vector.tensor_scalar(a0st[:], attn0[mi][:], SCALE_STAT,
                                    None, op0=OP.mult)
            nc.gpsimd.dma_start(
                a0_bounce[mi * 128:(mi + 1) * 128, :], a0st[:])
        nc.gpsimd.collective_compute(
            "AllGather", OP.bypass, replica_groups=GROUPS,
            ins=[a0_bounce[:, :]], outs=[a0_full[:, :]])

        # pass1 mid (covers the pooled1 AG)
        for kq in range(3, 6):
            sweep_block(kq, 0)

        with tc.tile_pool(name="pf1_pool", bufs=3) as pf1_pool:
            for jb in range(16):
                pt = pf1_pool.tile([128, C_], BF16, name="pf1_t", tag="pf1_t")
                nc.sync.dma_start(pt[:], pooled1_full[jb * 128:(jb + 1) * 128, :])
                _tp128(nc, psum_a,
                       pooled1_fullT[:, jb * 128:(jb + 1) * 128],
                       pt[:], ident_bf16[:], BF16, f"tp_pf1_{jb}")

        attn1 = _attention_step(nc, pools, pooled1T_loc[:], pooled1_fullT[:], 1)

        # pass1 tail (covers the attn1 search)
        for kq in range(6, 11):
            sweep_block(kq, 0)

        # a0_sw: reuse msrc for allgathered attn0 rows (issued mid-pass1
        # so the loads drain well before M4T)
        a0_sw = msrc_pool.tile([128, 16, NP_], F8, name="a0_sw_sb",
                               tag="msrc")
        nc.sync.dma_start(
            a0_sw[:], a0_full[:, :].rearrange("(mb p) j -> p mb j", p=128))
        msrc = a0_sw  # mT_block reads msrc

        for kq in range(11, 16):
            sweep_block(kq, 0)

        # attn1 -> at1T_sb (grouped transposes, as for attn0)
        at1_jbv = at1T_sb[:].rearrange("p (jb w) -> p jb w", w=256)
        for mi in range(2):
            for g in range(4):
                ps = psum_a.tile([128, 4, 128], BF16, name=f"tpa1_{mi}_{g}",
                                 tag="attn_ps")
                for j4 in range(4):
                    jb = g * 4 + j4
                    nc.tensor.transpose(
                        ps[:, j4, :],
                        attn1[mi][:, jb * 128:(jb + 1) * 128], ident_bf16[:])
                dst = at1_jbv[:, g * 4:(g + 1) * 4,
                              mi * 128:(mi + 1) * 128]
                nc.vector.tensor_scalar(dst, ps[:], SCALE_STAT, None,
                                        op0=OP.mult)

        # M4T
        mT_block(768, None, 1.0 / 512.0)

        # pass2 + fused LN
        for kq in range(16):
            sweep_block(kq, 1)

        p1a0_pool.release()
        ln_pool.release()
        xx_pool.release()
        xs_pool.release()
        psum_c.release()
        psum_b.release()
        big_u.release()
        big_lg.release()
        psum_a.release()
        msrc_pool.release()
        small_pool.release()
        const_pool.release()
        dram.release()

    nc.finalize()
    return nc


# ---------------------------------------------------------------------------
# Host side
# ---------------------------------------------------------------------------
_CACHE = {}


def _get_graph(trivial_affine, gate):
    key = (bool(trivial_affine), round(float(gate), 8))
    if key not in _CACHE:
        _CACHE[key] = build_graph(key[0], gate=key[1])
    return _CACHE[key]


def prepare_in_maps(x, prior, W1, W2, W3, prior_fwd_w, adaptive_w,
                    ln_gamma, ln_beta, alpha):
    bf = ml_dtypes.bfloat16
    f8 = ml_dtypes.float8_e4m3
    x2 = np.ascontiguousarray(np.asarray(x, np.float32).reshape(NP_, KC))
    x_f8 = x2.astype(f8)
    # xsw[kq*2+jh, p, jb*512+c] = x_f8[jh*1024+jb*128+p, kq*512+c]
    xsw = np.ascontiguousarray(
        x_f8.reshape(2, 8, 128, 16, 512).transpose(3, 0, 2, 1, 4).reshape(
            32, 128, 4096))
    pooled0 = np.asarray(x, np.float32).reshape(NP_, NC_, C_).mean(axis=1)
    pooled0T_bf = np.ascontiguousarray(pooled0.T).astype(bf)     # [C, NP]
    pooled0_f8 = (pooled0 * 8.0).astype(f8)                      # [NP, C]
    prior = np.asarray(prior, np.float32)
    rs = np.maximum(prior.sum(axis=1, keepdims=True), 1e-12)
    rf = (prior / rs).astype(np.float32)
    rf512 = (rf * 512.0).astype(f8)
    # rf_sw[p, mb, j] = (512*Rf)[mb*128+p, j]
    rf_sw = np.ascontiguousarray(
        rf512.reshape(16, 128, NP_).transpose(1, 0, 2))

    W1 = np.asarray(W1, np.float32)
    W2 = np.asarray(W2, np.float32)
    W3 = np.asarray(W3, np.float32)
    G = (W2 @ W1)                       # [D, C]
    gT_h = np.ascontiguousarray(G.T).astype(bf)       # [C, D]
    w3T_h = np.ascontiguousarray(W3.T).astype(bf)     # [C, D]

    pw = np.asarray(prior_fwd_w, np.float32)
    aw = np.asarray(adaptive_w, np.float32)
    wp0T = (np.ascontiguousarray(pw[0].T) * SCALE_W).astype(f8)
    wp1T = (np.ascontiguousarray(pw[1].T) * SCALE_W).astype(f8)
    wa0T = (np.ascontiguousarray(aw[0].T) * SCALE_W).astype(f8)
    wa1T = (np.ascontiguousarray(aw[1].T) * SCALE_W).astype(f8)
    wpa_f = np.ascontiguousarray(np.concatenate([wp0T, wp1T], axis=1))
    wpa_a = np.ascontiguousarray(np.concatenate([wa0T, wa1T], axis=1))

    gate = 1.0 / (1.0 + np.exp(-np.float32(np.asarray(alpha).reshape(-1)[0])))

    gamma = np.asarray(ln_gamma, np.float32)
    beta = np.asarray(ln_beta, np.float32)
    trivial_affine = bool(np.all(gamma == 1.0) and np.all(beta == 0.0))

    in_maps = []
    for c in range(NCORES):
        rows = slice(c * R_, (c + 1) * R_)
        m = {
            "xsw": xsw,
            "x_loc": x2[rows].astype(bf),
            "rfT_f8": np.ascontiguousarray(rf512[rows].T),
            "rf_sw": rf_sw,
            "gT": gT_h,
            "w3T": w3T_h,
            "pooled0T_bf": pooled0T_bf,
            "pooled0T_loc_bf": np.ascontiguousarray(pooled0T_bf[:, rows]),
            "pooled0_f8": pooled0_f8,
            "wpa_f": wpa_f, "wpa_a": wpa_a,
        }
        if not trivial_affine:
            m["gamma_rep"] = np.broadcast_to(gamma, (128, C_)).copy()
            m["beta_rep"] = np.broadcast_to(beta, (128, C_)).copy()
        in_maps.append(m)
    return in_maps, trivial_affine, gate


def run(x, prior, W1, W2, W3, prior_fwd_w, adaptive_w, ln_gamma, ln_beta,
        alpha, trace=False):
    in_maps, trivial_affine, gate = prepare_in_maps(
        x, prior, W1, W2, W3, prior_fwd_w, adaptive_w, ln_gamma, ln_beta, alpha)
    nc = _get_graph(trivial_affine, gate)
    res = run_bass_kernel_spmd(nc, in_maps, core_ids=list(range(NCORES)),
                               trace=trace)
    out = np.concatenate([np.asarray(res.results[c]["out_loc"]).astype(
        np.float32) for c in range(NCORES)], axis=0)
    return out.reshape(NP_, NC_, C_), res


def kernel(x, prior, W1, W2, W3, prior_fwd_w, adaptive_w, ln_gamma, ln_beta,
           alpha):
    out, _ = run(x, prior, W1, W2, W3, prior_fwd_w, adaptive_w, ln_gamma,
                 ln_beta, alpha, trace=False)
    return out
